# revision 3
# baseline (speedup 1.0000x reference)
"""DigitCaps u_hat kernel for Trainium2 (8 NeuronCores, SPMD).

Computes u_hat[b,r,c,o] = sum_i W[0,r,c,o,i] * x[b,r,i] + bias[o,0]
with B=512, R=1152, C=10, O=16, I=8 -> output [512, 1152, 10, 16, 1] f32.

Strategy (v2)
-------------
Shard R across the 8 cores: 144 r-values per core; each core writes its
[512, 144, 160] f16 output slice (23.6 MB — the kernel is output-DMA
bound at ~394 GB/s per core, so the whole game is starting that stream
early and never letting it stall).

Per group of G=3 r-values one matmul computes a [128 b, 480] tile:
  k = (r', i), i in [0,9)  (8 x-values + a constant-1 row for the bias)
  lhsT [27, 128] = x^T for a 128-wide b-block      (stationary)
  rhs  [27, 480] = block-diag W (3 x [9,160], bias row included)

v1 zero-padded K 27->128 (K<=32 matmuls stream slower), which required
zeroing every input tile: ~20 us of memset+drain serialized at kernel
start and delayed the first output DMA to t=24 us (trace-measured).
v2 instead uses PE row tiling: 4 groups ("a quad") sit at partition
offsets {0,32,64,96}; each matmul is K=27 in 32x128 tile mode, the four
stream concurrently through disjoint 32-row PE bands into 4 separate
PSUM banks, and the pad rows 27..32 of each band are simply never read
-- no memsets at all.  Inputs land as one contiguous [128, 2, 992] f16
DMA per chunk (full 16-port coverage vs 27-partition transfers in v1).

PSUM->SBUF evacuation alternates DVE/ACT per quad ([128,4,480] f32->f16
copies); output DMAs (2 quads = 0.98 MB) stream on the sync HWDGE ring.
"""

import numpy as np

# Problem constants (hardcoded per harness contract).
B, R, C, O, I = 512, 1152, 10, 16, 8
CO = C * O                      # 160
NCORES = 8
RS = R // NCORES                # 144 r per core
G = 3                           # r-values per matmul (block-diag pack)
K = G * (I + 1)                 # 27 contraction rows (incl. bias row)
BANDS = 4                       # row-tiled matmuls per quad (PE 32x128 mode)
QUADS = RS // (G * BANDS)       # 12 quads per core
QPC = 2                         # quads per input chunk
CHUNKS = QUADS // QPC           # 6 input chunks (for early compute start)
XC = B                          # 512 x columns per quad slot
WC = G * CO                     # 480 W columns per quad slot
TC = XC + WC                    # 992 packed input columns
DMA_Q = 2                       # quads per output DMA (~1 MB transfers)
BBLK = B // 128                 # 4 b-blocks

OP_DT = "f16"                   # operand dtype (kept for test.py compat)
OUT_DT = "f16"                  # device output dtype

_prog_cache = {}


def _build_program(op_dt=OP_DT, out_dt=OUT_DT):
    import concourse.bacc as bacc
    import concourse.tile as tile
    from concourse import mybir

    key = (op_dt, out_dt)
    if key in _prog_cache:
        return _prog_cache[key]

    f32 = mybir.dt.float32
    f16 = mybir.dt.float16

    # Bacc (not raw Bass): its finalize() runs move_matmul_waits_to_ldweights
    # + generate_event_semaphores, required to satisfy the per-instruction
    # sync-wait limits at codegen.
    nc = bacc.Bacc("TRN2", target_bir_lowering=False, debug=False)

    in_d = nc.declare_dram_parameter(
        "inp", [CHUNKS, 128, QPC, TC], f16, isOutput=False
    )
    out_d = nc.declare_dram_parameter("out", [B, RS, CO], f16, isOutput=True)

    with tile.TileContext(nc) as tc:
        with (
            tc.tile_pool(name="const", bufs=1) as const,
            tc.tile_pool(name="psum", bufs=2, space="PSUM") as psum,
            tc.tile_pool(name="outp", bufs=6) as outp,
        ):
            insb = []
            for ch in range(CHUNKS):
                t = const.tile([128, QPC, TC], f16, tag=f"in{ch}")
                # chunk 0 gates the first matmul: HWDGE (scalar ring) for
                # its ~0.6us first-byte latency; the rest stream via SWDGE
                # (gpsimd) so neither HWDGE ring's sequencer is tied up.
                if ch == 0:
                    nc.scalar.dma_start(out=t[:], in_=in_d[ch])
                else:
                    nc.gpsimd.dma_start(out=t[:], in_=in_d[ch])
                insb.append(t)

            cidx = 0
            for j in range(BBLK):
                for dq in range(QUADS // DMA_Q):
                    ot = outp.tile([128, DMA_Q, BANDS, WC], f16)
                    for s2 in range(DMA_Q):
                        q = dq * DMA_Q + s2
                        ch, s = divmod(q, QPC)
                        ps = psum.tile([128, BANDS, 512], f32)
                        for band in range(BANDS):
                            pb = 32 * band
                            lhsT = insb[ch][pb : pb + K, s, j * 128 : (j + 1) * 128]
                            rhs = insb[ch][pb : pb + K, s, XC : XC + WC]
                            # K=27 at partition offset pb: 32x128 row-tile
                            # mode, 4 concurrent streams into banks 0..3.
                            # Explicit tile_position: auto-derive rejects
                            # base partition 96.
                            nc.tensor.matmul(
                                ps[:, band, 0:WC], lhsT, rhs,
                                start=True, stop=True,
                                tile_position=(pb, 0),
                            )
                        # Alternate evacuation between DVE and ACT: the two
                        # [128,4,480] f32->f16 copies of one output tile run
                        # concurrently on the two engines.
                        if cidx % 2 == 0:
                            nc.vector.tensor_copy(ot[:, s2], ps[:, :, 0:WC])
                        else:
                            nc.scalar.copy(ot[:, s2], ps[:, :, 0:WC])
                        cidx += 1
                    nc.sync.dma_start(
                        out=out_d[
                            j * 128 : (j + 1) * 128,
                            dq * DMA_Q * G * BANDS : (dq + 1) * DMA_Q * G * BANDS,
                            :,
                        ],
                        in_=ot[:],
                    )

    nc.finalize()
    _prog_cache[key] = nc
    return nc


def _prep_inputs(x, W, bias, op_dt=OP_DT):
    """Build per-core packed input arrays in the device layout.

    Per chunk: [128, QPC, TC] f16 where partition p = 32*band + k,
    k = r'*9 + i (i=8 is the constant-1 bias row; rows 27..32 are pad),
    slot s picks the quad, cols [0:512] = x^T (b), cols [512:992] = the
    [27, 480] block-diag W for the band's group.
    """
    x = np.ascontiguousarray(x, dtype=np.float32)
    W = np.ascontiguousarray(W, dtype=np.float32)
    bias = np.ascontiguousarray(bias, dtype=np.float32)

    Wf = W[0].reshape(R, CO, I)                      # [R, CO, I]
    bias_co = np.tile(bias[:, 0], C)                 # [CO]
    NG = RS // G                                     # 48 groups per core

    in_maps = []
    for c in range(NCORES):
        r0 = c * RS
        arr = np.zeros((CHUNKS, BANDS, 32, QPC, TC), dtype=np.float16)

        xT = x[:, r0 : r0 + RS, :].transpose(1, 2, 0)    # [RS, I, B]
        seg9 = np.empty((RS, I + 1, B), dtype=np.float32)
        seg9[:, :I] = xT
        seg9[:, I] = 1.0
        g27 = seg9.reshape(NG, K, B)                     # rows k = r'*9+i
        # group g = (ch*QPC + s)*BANDS + band
        arr[:, :, :K, :, :XC] = (
            g27.reshape(CHUNKS, QPC, BANDS, K, B).transpose(0, 2, 3, 1, 4)
        )

        Wc = Wf[r0 : r0 + RS]                            # [RS, CO, I]
        W9 = np.empty((RS, I + 1, CO), dtype=np.float32)
        W9[:, :I] = Wc.transpose(0, 2, 1)
        W9[:, I] = bias_co
        blk = np.zeros((NG, G, I + 1, G, CO), dtype=np.float32)
        W9g = W9.reshape(NG, G, I + 1, CO)
        for rp in range(G):
            blk[:, rp, :, rp, :] = W9g[:, rp]
        blk27 = blk.reshape(NG, K, WC)
        arr[:, :, :K, :, XC:] = (
            blk27.reshape(CHUNKS, QPC, BANDS, K, WC).transpose(0, 2, 3, 1, 4)
        )

        in_maps.append({"inp": arr.reshape(CHUNKS, 128, QPC, TC)})
    return in_maps


def _run(inputs, trace=False, op_dt=OP_DT, out_dt=OUT_DT, **kw):
    from concourse.bass_utils import run_bass_kernel_spmd

    nc = _build_program(op_dt, out_dt)
    in_maps = _prep_inputs(inputs["x"], inputs["W"], inputs["bias"], op_dt)
    res = run_bass_kernel_spmd(
        nc, in_maps, list(range(NCORES)), trace=trace, **kw
    )
    outs = [np.asarray(res.results[c]["out"]) for c in range(NCORES)]
    full = np.concatenate(outs, axis=1)               # [B, R, CO]
    full = full.astype(np.float32, copy=False)
    return np.ascontiguousarray(full).reshape(B, R, C, O, 1), res


def kernel(x, W, bias):
    out, _ = _run({"x": x, "W": W, "bias": bias})
    return out


# revision 4
# speedup vs baseline: 1.0237x; 1.0237x over previous
"""DigitCaps u_hat kernel for Trainium2 (8 NeuronCores, SPMD).

Computes u_hat[b,r,c,o] = sum_i W[0,r,c,o,i] * x[b,r,i] + bias[o,0]
with B=512, R=1152, C=10, O=16, I=8 -> output [512, 1152, 10, 16, 1] f32.

Strategy (v2)
-------------
Shard R across the 8 cores: 144 r-values per core; each core writes its
[512, 144, 160] f16 output slice (23.6 MB — the kernel is output-DMA
bound at ~394 GB/s per core, so the whole game is starting that stream
early and never letting it stall).

Per group of G=3 r-values one matmul computes a [128 b, 480] tile:
  k = (r', i), i in [0,9)  (8 x-values + a constant-1 row for the bias)
  lhsT [27, 128] = x^T for a 128-wide b-block      (stationary)
  rhs  [27, 480] = block-diag W (3 x [9,160], bias row included)

v1 zero-padded K 27->128 (K<=32 matmuls stream slower), which required
zeroing every input tile: ~20 us of memset+drain serialized at kernel
start and delayed the first output DMA to t=24 us (trace-measured).
v2 instead uses PE row tiling: 4 groups ("a quad") sit at partition
offsets {0,32,64,96}; each matmul is K=27 in 32x128 tile mode, the four
stream concurrently through disjoint 32-row PE bands into 4 separate
PSUM banks, and the pad rows 27..32 of each band are simply never read
-- no memsets at all.  Inputs land as one contiguous [128, 2, 992] f16
DMA per chunk (full 16-port coverage vs 27-partition transfers in v1).

PSUM->SBUF evacuation alternates DVE/ACT per quad ([128,4,480] f32->f16
copies); output DMAs (2 quads = 0.98 MB) stream on the sync HWDGE ring.
"""

import numpy as np

# Problem constants (hardcoded per harness contract).
B, R, C, O, I = 512, 1152, 10, 16, 8
CO = C * O                      # 160
NCORES = 8
RS = R // NCORES                # 144 r per core
G = 3                           # r-values per matmul (block-diag pack)
K = G * (I + 1)                 # 27 contraction rows (incl. bias row)
BANDS = 4                       # row-tiled matmuls per quad (PE 32x128 mode)
QUADS = RS // (G * BANDS)       # 12 quads per core
QPC = 2                         # quads per input chunk
CHUNKS = QUADS // QPC           # 6 input chunks (for early compute start)
XC = B                          # 512 x columns per quad slot
WC = G * CO                     # 480 W columns per quad slot
TC = XC + WC                    # 992 packed input columns
DMA_Q = 2                       # quads per output DMA (~1 MB transfers)
BBLK = B // 128                 # 4 b-blocks

OP_DT = "f16"                   # operand dtype (kept for test.py compat)
OUT_DT = "f16"                  # device output dtype

_prog_cache = {}


def _build_program(op_dt=OP_DT, out_dt=OUT_DT):
    import concourse.bacc as bacc
    import concourse.tile as tile
    from concourse import mybir

    key = (op_dt, out_dt)
    if key in _prog_cache:
        return _prog_cache[key]

    f32 = mybir.dt.float32
    f16 = mybir.dt.float16

    # Bacc (not raw Bass): its finalize() runs move_matmul_waits_to_ldweights
    # + generate_event_semaphores, required to satisfy the per-instruction
    # sync-wait limits at codegen.
    nc = bacc.Bacc("TRN2", target_bir_lowering=False, debug=False)

    in_d = nc.declare_dram_parameter(
        "inp", [CHUNKS, 128, QPC, TC], f16, isOutput=False
    )
    out_d = nc.declare_dram_parameter("out", [B, RS, CO], f16, isOutput=True)

    with tile.TileContext(nc) as tc:
        with (
            tc.tile_pool(name="const", bufs=1) as const,
            tc.tile_pool(name="psum", bufs=4, space="PSUM") as psum,
            tc.tile_pool(name="outp", bufs=8) as outp,
        ):
            insb = []
            for ch in range(CHUNKS):
                t = const.tile([128, QPC, TC], f16, tag=f"in{ch}")
                # All input loads on the scalar HWDGE ring: one FIFO queue
                # drains chunk 0 first at full rate (a second queue would
                # steal SDMA bandwidth from the load that gates the first
                # matmul).  Chunk 0 is split per quad so quad 0 lands ASAP.
                if ch == 0:
                    for s in range(QPC):
                        nc.scalar.dma_start(out=t[:, s], in_=in_d[ch, :, s])
                else:
                    nc.scalar.dma_start(out=t[:], in_=in_d[ch])
                insb.append(t)

            for j in range(BBLK):
                for q in range(QUADS):
                    ch, s = divmod(q, QPC)
                    ot = outp.tile([128, BANDS, WC], f16)
                    # Two 2-bank psum tiles per quad, evacuated by the two
                    # engines in parallel; with bufs=4 the copy->matmul
                    # reuse chain spans two quads, so neither the matmuls
                    # nor the other engine's copy sit on the critical path.
                    for h in range(2):
                        ps = psum.tile([128, 2, 512], f32)
                        for b2 in range(2):
                            band = 2 * h + b2
                            pb = 32 * band
                            lhsT = insb[ch][pb : pb + K, s, j * 128 : (j + 1) * 128]
                            rhs = insb[ch][pb : pb + K, s, XC : XC + WC]
                            # K=27 at partition offset pb: 32x128 row-tile
                            # mode, 4 concurrent streams into 4 banks.
                            # Explicit tile_position: auto-derive rejects
                            # base partition 96.
                            nc.tensor.matmul(
                                ps[:, b2, 0:WC], lhsT, rhs,
                                start=True, stop=True,
                                tile_position=(pb, 0),
                            )
                        if h == 0:
                            nc.vector.tensor_copy(
                                ot[:, 0:2, :], ps[:, :, 0:WC]
                            )
                        else:
                            nc.scalar.copy(ot[:, 2:4, :], ps[:, :, 0:WC])
                    nc.sync.dma_start(
                        out=out_d[
                            j * 128 : (j + 1) * 128,
                            q * G * BANDS : (q + 1) * G * BANDS,
                            :,
                        ],
                        in_=ot[:],
                    )

    nc.finalize()
    _prog_cache[key] = nc
    return nc


def _prep_inputs(x, W, bias, op_dt=OP_DT):
    """Build per-core packed input arrays in the device layout.

    Per chunk: [128, QPC, TC] f16 where partition p = 32*band + k,
    k = r'*9 + i (i=8 is the constant-1 bias row; rows 27..32 are pad),
    slot s picks the quad, cols [0:512] = x^T (b), cols [512:992] = the
    [27, 480] block-diag W for the band's group.
    """
    x = np.ascontiguousarray(x, dtype=np.float32)
    W = np.ascontiguousarray(W, dtype=np.float32)
    bias = np.ascontiguousarray(bias, dtype=np.float32)

    Wf = W[0].reshape(R, CO, I)                      # [R, CO, I]
    bias_co = np.tile(bias[:, 0], C)                 # [CO]
    NG = RS // G                                     # 48 groups per core

    in_maps = []
    for c in range(NCORES):
        r0 = c * RS
        arr = np.zeros((CHUNKS, BANDS, 32, QPC, TC), dtype=np.float16)

        xT = x[:, r0 : r0 + RS, :].transpose(1, 2, 0)    # [RS, I, B]
        seg9 = np.empty((RS, I + 1, B), dtype=np.float32)
        seg9[:, :I] = xT
        seg9[:, I] = 1.0
        g27 = seg9.reshape(NG, K, B)                     # rows k = r'*9+i
        # group g = (ch*QPC + s)*BANDS + band
        arr[:, :, :K, :, :XC] = (
            g27.reshape(CHUNKS, QPC, BANDS, K, B).transpose(0, 2, 3, 1, 4)
        )

        Wc = Wf[r0 : r0 + RS]                            # [RS, CO, I]
        W9 = np.empty((RS, I + 1, CO), dtype=np.float32)
        W9[:, :I] = Wc.transpose(0, 2, 1)
        W9[:, I] = bias_co
        blk = np.zeros((NG, G, I + 1, G, CO), dtype=np.float32)
        W9g = W9.reshape(NG, G, I + 1, CO)
        for rp in range(G):
            blk[:, rp, :, rp, :] = W9g[:, rp]
        blk27 = blk.reshape(NG, K, WC)
        arr[:, :, :K, :, XC:] = (
            blk27.reshape(CHUNKS, QPC, BANDS, K, WC).transpose(0, 2, 3, 1, 4)
        )

        in_maps.append({"inp": arr.reshape(CHUNKS, 128, QPC, TC)})
    return in_maps


def _run(inputs, trace=False, op_dt=OP_DT, out_dt=OUT_DT, **kw):
    from concourse.bass_utils import run_bass_kernel_spmd

    nc = _build_program(op_dt, out_dt)
    in_maps = _prep_inputs(inputs["x"], inputs["W"], inputs["bias"], op_dt)
    res = run_bass_kernel_spmd(
        nc, in_maps, list(range(NCORES)), trace=trace, **kw
    )
    outs = [np.asarray(res.results[c]["out"]) for c in range(NCORES)]
    full = np.concatenate(outs, axis=1)               # [B, R, CO]
    full = full.astype(np.float32, copy=False)
    return np.ascontiguousarray(full).reshape(B, R, C, O, 1), res


def kernel(x, W, bias):
    out, _ = _run({"x": x, "W": W, "bias": bias})
    return out


# revision 6
# speedup vs baseline: 1.1889x; 1.1614x over previous
"""DigitCaps u_hat kernel for Trainium2 (8 NeuronCores, SPMD).

Computes u_hat[b,r,c,o] = sum_i W[0,r,c,o,i] * x[b,r,i] + bias[o,0]
with B=512, R=1152, C=10, O=16, I=8 -> output [512, 1152, 10, 16, 1] f32.

Strategy (v2)
-------------
Shard R across the 8 cores: 144 r-values per core; each core writes its
[512, 144, 160] f16 output slice (23.6 MB — the kernel is output-DMA
bound at ~394 GB/s per core, so the whole game is starting that stream
early and never letting it stall).

Per group of G=3 r-values one matmul computes a [128 b, 480] tile:
  k = (r', i), i in [0,9)  (8 x-values + a constant-1 row for the bias)
  lhsT [27, 128] = x^T for a 128-wide b-block      (stationary)
  rhs  [27, 480] = block-diag W (3 x [9,160], bias row included)

v1 zero-padded K 27->128 (K<=32 matmuls stream slower), which required
zeroing every input tile: ~20 us of memset+drain serialized at kernel
start and delayed the first output DMA to t=24 us (trace-measured).
v2 instead uses PE row tiling: 4 groups ("a quad") sit at partition
offsets {0,32,64,96}; each matmul is K=27 in 32x128 tile mode, the four
stream concurrently through disjoint 32-row PE bands into 4 separate
PSUM banks, and the pad rows 27..32 of each band are simply never read
-- no memsets at all.  Inputs land as one contiguous [128, 2, 992] f16
DMA per chunk (full 16-port coverage vs 27-partition transfers in v1).

PSUM->SBUF evacuation alternates DVE/ACT per quad ([128,4,480] f32->f16
copies); output DMAs (2 quads = 0.98 MB) stream on the sync HWDGE ring.
"""

import numpy as np

# Problem constants (hardcoded per harness contract).
B, R, C, O, I = 512, 1152, 10, 16, 8
CO = C * O                      # 160
NCORES = 8
RS = R // NCORES                # 144 r per core
G = 3                           # r-values per matmul (block-diag pack)
K = G * (I + 1)                 # 27 contraction rows (incl. bias row)
BANDS = 4                       # row-tiled matmuls per quad (PE 32x128 mode)
QUADS = RS // (G * BANDS)       # 12 quads per core
QPC = 2                         # quads per input chunk
CHUNKS = QUADS // QPC           # 6 input chunks (for early compute start)
XC = B                          # 512 x columns per quad slot
WC = G * CO                     # 480 W columns per quad slot
TC = XC + WC                    # 992 packed input columns
DMA_Q = 2                       # quads per output DMA (~1 MB transfers)
BBLK = B // 128                 # 4 b-blocks

OP_DT = "f16"                   # operand dtype (kept for test.py compat)
OUT_DT = "f16"                  # device output dtype

_prog_cache = {}


def _build_program(op_dt=OP_DT, out_dt=OUT_DT):
    import concourse.bacc as bacc
    import concourse.tile as tile
    from concourse import mybir

    key = (op_dt, out_dt)
    if key in _prog_cache:
        return _prog_cache[key]

    f32 = mybir.dt.float32
    f16 = mybir.dt.float16

    # Bacc (not raw Bass): its finalize() runs move_matmul_waits_to_ldweights
    # + generate_event_semaphores, required to satisfy the per-instruction
    # sync-wait limits at codegen.
    nc = bacc.Bacc("TRN2", target_bir_lowering=False, debug=False)

    in_d = nc.declare_dram_parameter(
        "inp", [CHUNKS, 128, QPC, TC], f16, isOutput=False
    )
    out_d = nc.declare_dram_parameter("out", [B, RS, CO], f16, isOutput=True)

    with tile.TileContext(nc) as tc:
        with (
            tc.tile_pool(name="const", bufs=1) as const,
            tc.tile_pool(name="psum", bufs=2, space="PSUM") as psum,
            tc.tile_pool(name="outp", bufs=4) as outp,
        ):
            insb = []
            for ch in range(CHUNKS):
                t = const.tile([128, QPC, TC], f16, tag=f"in{ch}")
                # All input loads on the gpsimd SWDGE queue: one FIFO queue
                # drains chunk 0 first at full rate, and neither HWDGE
                # sequencer (ACT: copies, SP: output DMAs) is tied up with
                # input issues.  Chunk 0 is split per quad so quad 0 lands
                # ASAP.
                if ch == 0:
                    for s in range(QPC):
                        nc.gpsimd.dma_start(out=t[:, s], in_=in_d[ch, :, s])
                else:
                    nc.gpsimd.dma_start(out=t[:], in_=in_d[ch])
                insb.append(t)

            for j in range(BBLK):
                for dq in range(QUADS // DMA_Q):
                    ot = outp.tile([128, DMA_Q, BANDS, WC], f16)
                    for s2 in range(DMA_Q):
                        q = dq * DMA_Q + s2
                        ch, s = divmod(q, QPC)
                        # Two 2-bank psum tiles per quad with DEDICATED tag
                        # rings (bufs=2 each): tile reuse chains A(m)->A(m-2)
                        # give two quads of slack, so neither the matmuls nor
                        # the copies ever wait on the previous quad (the
                        # untagged pool interleaves A/B in one ring, which
                        # serializes quad m's matmuls behind quad m-1's
                        # copies -- measured 1.47us/quad vs the 1.25us DMA
                        # floor).
                        for h in range(2):
                            ps = psum.tile(
                                [128, 2, 512], f32, tag=f"ps{h}", bufs=2
                            )
                            for b2 in range(2):
                                band = 2 * h + b2
                                pb = 32 * band
                                lhsT = insb[ch][
                                    pb : pb + K, s, j * 128 : (j + 1) * 128
                                ]
                                rhs = insb[ch][pb : pb + K, s, XC : XC + WC]
                                # K=27 at partition offset pb: 32x128
                                # row-tile mode, 4 concurrent streams into 4
                                # banks.  Explicit tile_position: auto-derive
                                # rejects base partition 96.
                                nc.tensor.matmul(
                                    ps[:, b2, 0:WC], lhsT, rhs,
                                    start=True, stop=True,
                                    tile_position=(pb, 0),
                                )
                            if h == 0:
                                nc.vector.tensor_copy(
                                    ot[:, s2, 0:2, :], ps[:, :, 0:WC]
                                )
                            else:
                                nc.scalar.copy(
                                    ot[:, s2, 2:4, :], ps[:, :, 0:WC]
                                )
                    nc.sync.dma_start(
                        out=out_d[
                            j * 128 : (j + 1) * 128,
                            dq * DMA_Q * G * BANDS : (dq + 1) * DMA_Q * G * BANDS,
                            :,
                        ],
                        in_=ot[:],
                    )

    nc.finalize()
    _prog_cache[key] = nc
    return nc


def _prep_inputs(x, W, bias, op_dt=OP_DT):
    """Build per-core packed input arrays in the device layout.

    Per chunk: [128, QPC, TC] f16 where partition p = 32*band + k,
    k = r'*9 + i (i=8 is the constant-1 bias row; rows 27..32 are pad),
    slot s picks the quad, cols [0:512] = x^T (b), cols [512:992] = the
    [27, 480] block-diag W for the band's group.
    """
    x = np.ascontiguousarray(x, dtype=np.float32)
    W = np.ascontiguousarray(W, dtype=np.float32)
    bias = np.ascontiguousarray(bias, dtype=np.float32)

    Wf = W[0].reshape(R, CO, I)                      # [R, CO, I]
    bias_co = np.tile(bias[:, 0], C)                 # [CO]
    NG = RS // G                                     # 48 groups per core

    in_maps = []
    for c in range(NCORES):
        r0 = c * RS
        arr = np.zeros((CHUNKS, BANDS, 32, QPC, TC), dtype=np.float16)

        xT = x[:, r0 : r0 + RS, :].transpose(1, 2, 0)    # [RS, I, B]
        seg9 = np.empty((RS, I + 1, B), dtype=np.float32)
        seg9[:, :I] = xT
        seg9[:, I] = 1.0
        g27 = seg9.reshape(NG, K, B)                     # rows k = r'*9+i
        # group g = (ch*QPC + s)*BANDS + band
        arr[:, :, :K, :, :XC] = (
            g27.reshape(CHUNKS, QPC, BANDS, K, B).transpose(0, 2, 3, 1, 4)
        )

        Wc = Wf[r0 : r0 + RS]                            # [RS, CO, I]
        W9 = np.empty((RS, I + 1, CO), dtype=np.float32)
        W9[:, :I] = Wc.transpose(0, 2, 1)
        W9[:, I] = bias_co
        blk = np.zeros((NG, G, I + 1, G, CO), dtype=np.float32)
        W9g = W9.reshape(NG, G, I + 1, CO)
        for rp in range(G):
            blk[:, rp, :, rp, :] = W9g[:, rp]
        blk27 = blk.reshape(NG, K, WC)
        arr[:, :, :K, :, XC:] = (
            blk27.reshape(CHUNKS, QPC, BANDS, K, WC).transpose(0, 2, 3, 1, 4)
        )

        in_maps.append({"inp": arr.reshape(CHUNKS, 128, QPC, TC)})
    return in_maps


def _run(inputs, trace=False, op_dt=OP_DT, out_dt=OUT_DT, **kw):
    from concourse.bass_utils import run_bass_kernel_spmd

    nc = _build_program(op_dt, out_dt)
    in_maps = _prep_inputs(inputs["x"], inputs["W"], inputs["bias"], op_dt)
    res = run_bass_kernel_spmd(
        nc, in_maps, list(range(NCORES)), trace=trace, **kw
    )
    outs = [np.asarray(res.results[c]["out"]) for c in range(NCORES)]
    full = np.concatenate(outs, axis=1)               # [B, R, CO]
    full = full.astype(np.float32, copy=False)
    return np.ascontiguousarray(full).reshape(B, R, C, O, 1), res


def kernel(x, W, bias):
    out, _ = _run({"x": x, "W": W, "bias": bias})
    return out
